# revision 11
# baseline (speedup 1.0000x reference)
"""Fused MultiHeadAttention block (LN -> QKV -> attention -> proj -> +residual)
for Trainium2, sharded over 8 NeuronCores.

Sharding: data parallel over batch (B=2) x tensor parallel over heads
(16 heads -> 4 groups of 4). Core c handles batch c//4, head group c%4.
Each core computes a partial projection output (contraction over its 256
hd-dims) plus 0.25*(x + b_proj); the host sums the 4 partials per batch.

Math notes (all LN folding is exact linear algebra, valid for any
gamma/beta):
  h = LN(x)*gamma + beta;  qkv = h @ W
  With W' = diag(gamma) @ W, bW = beta @ W, rs[t] = 1/sqrt(var[t]+eps):
    qkv[t,d] = rs[t] * ( (x@W')[t,d] + (-mu[t])*colsum(W')[d]
                         + sd[t]*bW[d] ),   sd[t] = 1/rs[t]
  The two rank-1 terms are added in PSUM via K=1 matmuls; the rs[t] scale
  is applied at PSUM->SBUF drain time (per-partition for V whose rows are
  tokens; via a broadcast tile for Q; folded into the softmax exp scale
  for K, since exp(score_true) = exp(0.125*rs_k * psum_score) when Q
  already carries its rs factor).

  Softmax skips the max-subtraction (scores here are O(1), exp is safe);
  denominators come for free as row 64 of the AV matmul by appending a
  ones column to V (lhsT [k,65] -> out rows 0..63 = attn@V, row 64 =
  sum(exp)). Normalization folds into the attention-output tiles before
  the projection matmul.

Matmuls run in float32r (full PE rate at free-dim>=256, ~1e-3 rel err);
everything else is f32.
"""

import numpy as np

import concourse.bass as bass
import concourse.tile as tile
from concourse import bacc, mybir

F32 = mybir.dt.float32
F32R = mybir.dt.float32r
AF = mybir.ActivationFunctionType
ALU = mybir.AluOpType

B, N, C = 2, 2048, 1024
HEADS, HD = 16, 64
NCORES = 8
HG = 4                      # head groups (tensor-parallel degree)
H_PER = HEADS // HG         # 4 heads per core
CORE_HD = H_PER * HD        # 256 hd-dims per core
KT = C // 128               # 8 contraction tiles over channels
QT = 4                      # token tiles of 512
TB = N // 128               # 16 token blocks of 128
EPS = 1e-5


def build_nc(debug=False):
    nc = bacc.Bacc("TRN2", target_bir_lowering=False, debug=False,
                   num_devices=NCORES)

    xT = nc.dram_tensor("xT", [C, N], F32, kind="ExternalInput")
    xn = nc.dram_tensor("xn", [N, C], F32, kind="ExternalInput")
    wqk = nc.dram_tensor("wqk", [C, 512], F32, kind="ExternalInput")
    wv = nc.dram_tensor("wv", [C, 256], F32, kind="ExternalInput")
    wqksum = nc.dram_tensor("wqksum", [1, 512], F32, kind="ExternalInput")
    wqkbias = nc.dram_tensor("wqkbias", [1, 512], F32, kind="ExternalInput")
    wvsum = nc.dram_tensor("wvsum", [1, 256], F32, kind="ExternalInput")
    wvbias = nc.dram_tensor("wvbias", [1, 256], F32, kind="ExternalInput")
    wproj = nc.dram_tensor("wproj", [CORE_HD, C], F32, kind="ExternalInput")
    bproj4 = nc.dram_tensor("bproj4", [1, C], F32, kind="ExternalInput")
    onesr = nc.dram_tensor("onesr", [1, 128], F32, kind="ExternalInput")
    onesv = nc.dram_tensor("onesv", [128, TB, 4, 1], F32, kind="ExternalInput")
    out = nc.dram_tensor("out", [N, C], F32, kind="ExternalOutput")
    if debug:
        dbg_q = nc.dram_tensor("dbg_q", [128, 2, N], F32, kind="ExternalOutput")
        dbg_k = nc.dram_tensor("dbg_k", [128, 2, N], F32, kind="ExternalOutput")
        dbg_v = nc.dram_tensor("dbg_v", [128, TB, 260], F32, kind="ExternalOutput")
        dbg_at = nc.dram_tensor("dbg_at", [128, 2, N], F32, kind="ExternalOutput")
        dbg_st = nc.dram_tensor("dbg_st", [128, 4 * TB], F32, kind="ExternalOutput")
        dbg_rows = nc.dram_tensor("dbg_rows", [3, N], F32, kind="ExternalOutput")
        dbg_sc = nc.dram_tensor("dbg_sc", [128, 512], F32, kind="ExternalOutput")
        dbg_ex = nc.dram_tensor("dbg_ex", [128, 512], F32, kind="ExternalOutput")
        dbg_av = nc.dram_tensor("dbg_av", [65, 512], F32, kind="ExternalOutput")
        dbg_rr = nc.dram_tensor("dbg_rr", [1, 512], F32, kind="ExternalOutput")
        dbg_rb = nc.dram_tensor("dbg_rb", [64, 512], F32, kind="ExternalOutput")

    with tile.TileContext(nc) as tc:
        with (
            tc.tile_pool(name="consts", bufs=1) as consts,
            tc.tile_pool(name="wpool", bufs=1) as wpool,
            tc.tile_pool(name="xtp", bufs=2) as xtp,
            tc.tile_pool(name="xnp", bufs=2) as xnp,
            tc.tile_pool(name="stats", bufs=1) as stats,
            tc.tile_pool(name="mvp", bufs=4) as mvp,
            tc.tile_pool(name="qkv", bufs=1) as qkvp,
            tc.tile_pool(name="att", bufs=1) as attp,
            tc.tile_pool(name="exps", bufs=4) as expp,
            tc.tile_pool(name="rec", bufs=4) as recp,
            tc.tile_pool(name="rbc", bufs=4) as rbcp,
            tc.tile_pool(name="outs", bufs=3) as outp,
            tc.tile_pool(name="dram", bufs=1, space="DRAM") as dramp,
            tc.tile_pool(name="psmm", bufs=2, space="PSUM") as psmm,
            tc.tile_pool(name="pssc", bufs=4, space="PSUM") as pssc,
            tc.tile_pool(name="psav", bufs=2, space="PSUM") as psav,
        ):
            # ---- constants / weights --------------------------------------
            wqk_sb = wpool.tile([128, KT, 512], F32R, tag="wbig")
            for k in range(KT):
                nc.sync.dma_start(
                    wqk_sb[:, k, :],
                    wqk[k * 128:(k + 1) * 128, :].bitcast(F32R))
            wv_sb = consts.tile([128, KT, 256], F32R)
            for k in range(KT):
                nc.sync.dma_start(
                    wv_sb[:, k, :],
                    wv[k * 128:(k + 1) * 128, :].bitcast(F32R))
            wqksum_sb = consts.tile([1, 512], F32R)
            nc.sync.dma_start(wqksum_sb[:], wqksum[:, :].bitcast(F32R))
            wqkbias_sb = consts.tile([1, 512], F32R)
            nc.sync.dma_start(wqkbias_sb[:], wqkbias[:, :].bitcast(F32R))
            wvsum_sb = consts.tile([1, 256], F32R)
            nc.sync.dma_start(wvsum_sb[:], wvsum[:, :].bitcast(F32R))
            wvbias_sb = consts.tile([1, 256], F32R)
            nc.sync.dma_start(wvbias_sb[:], wvbias[:, :].bitcast(F32R))
            bproj4_sb = consts.tile([1, C], F32R)
            nc.sync.dma_start(bproj4_sb[:], bproj4[:, :].bitcast(F32R))
            ones_sb = consts.tile([1, 128], F32R)
            nc.sync.dma_start(ones_sb[:], onesr[:, :].bitcast(F32R))
            eps_sb = consts.tile([128, 1], F32)
            nc.vector.memset(eps_sb[:], EPS)

            # ---- stage A: LN stats from natural-layout x ------------------
            a_col = stats.tile([128, TB], F32)    # rs[t] = 1/sqrt(var+eps)
            sd_col = stats.tile([128, TB], F32)   # sqrt(var+eps)
            nmu_col = stats.tile([128, TB], F32)  # -mean
            sc_col = stats.tile([128, TB], F32)   # 0.125 * rs  (exp scale)
            for tb in range(TB):
                xn_t = xnp.tile([128, C], F32, tag="xn")
                nc.sync.dma_start(xn_t[:], xn[tb * 128:(tb + 1) * 128, :])
                st = mvp.tile([128, 2, 6], F32, tag="bnst")
                xg = xn_t[:].rearrange("p (g d) -> p g d", g=2)
                for g in range(2):
                    nc.vector.bn_stats(st[:, g, :], xg[:, g, :])
                mv = mvp.tile([128, 2], F32, tag="bnmv")
                nc.vector.bn_aggr(mv[:], st[:])
                nc.scalar.activation(sd_col[:, tb:tb + 1], mv[:, 1:2],
                                     AF.Sqrt, bias=eps_sb[:])
                nc.vector.reciprocal(a_col[:, tb:tb + 1], sd_col[:, tb:tb + 1])
                nc.vector.tensor_scalar_mul(nmu_col[:, tb:tb + 1],
                                            mv[:, 0:1], -1.0)
            nc.vector.tensor_scalar_mul(sc_col[:], a_col[:], 0.125)

            # cols -> DRAM rows -> row/broadcast tiles
            rows3 = dramp.tile([3, N], F32)
            for i, col in enumerate((a_col, nmu_col, sd_col)):
                nc.sync.dma_start(
                    rows3[i:i + 1, :].rearrange("o (t p) -> o p t", p=128),
                    col[:, :])
            nmu_row = consts.tile([1, N], F32R)
            nc.sync.dma_start(nmu_row[:], rows3[1:2, :].bitcast(F32R))
            sd_row = consts.tile([1, N], F32R)
            nc.sync.dma_start(sd_row[:], rows3[2:3, :].bitcast(F32R))
            a_bc = consts.tile([128, N], F32)
            nc.sync.dma_start(a_bc[:], rows3[0:1, :].partition_broadcast(128))

            # ---- stages B/C: QKV projections ------------------------------
            q_sb = qkvp.tile([128, 2, N], F32R)   # [d-of-pair, pair, token]
            k_sb = qkvp.tile([128, 2, N], F32R)
            v_sb = qkvp.tile([128, TB, 4 * 65], F32R)  # [token, tb, h*65+d]
            v_view = v_sb[:].rearrange("p t (h c) -> p t h c", c=65)
            nc.sync.dma_start(v_view[:, :, :, 64:65],
                              onesv[:, :, :, :].bitcast(F32R))

            for qt in range(QT):
                ts = slice(qt * 512, (qt + 1) * 512)
                xT_t = xtp.tile([128, KT, 512], F32R, tag="xT")
                for k in range(KT):
                    nc.sync.dma_start(
                        xT_t[:, k, :],
                        xT[k * 128:(k + 1) * 128, ts].bitcast(F32R))
                # Q/K: out [128 dims(2 heads), 512 tokens]
                for dblk in range(4):
                    ds = slice(dblk * 128, (dblk + 1) * 128)
                    ps = psmm.tile([128, 512], F32, tag="mm")
                    for k in range(KT):
                        nc.tensor.matmul(ps[:], wqk_sb[:, k, ds], xT_t[:, k, :],
                                         start=(k == 0), stop=False)
                    nc.tensor.matmul(ps[:], wqksum_sb[:, ds], nmu_row[:, ts],
                                     start=False, stop=False)
                    nc.tensor.matmul(ps[:], wqkbias_sb[:, ds], sd_row[:, ts],
                                     start=False, stop=True)
                    dst = q_sb if dblk < 2 else k_sb
                    nc.vector.tensor_mul(dst[:, dblk % 2, ts], ps[:],
                                         a_bc[:, ts])
                # V: out [128 tokens, 256 dims]
                for t4 in range(4):
                    tb = qt * 4 + t4
                    bs = slice(tb * 128, (tb + 1) * 128)
                    ps = psmm.tile([128, 256], F32, tag="mm")
                    for k in range(KT):
                        nc.tensor.matmul(ps[:], xT_t[:, k, t4 * 128:(t4 + 1) * 128],
                                         wv_sb[:, k, :], start=(k == 0), stop=False)
                    nc.tensor.matmul(ps[:], nmu_row[:, bs], wvsum_sb[:],
                                     start=False, stop=False)
                    nc.tensor.matmul(ps[:], sd_row[:, bs], wvbias_sb[:],
                                     start=False, stop=True)
                    nc.scalar.activation(
                        v_view[:, tb, :, 0:64],
                        ps[:].rearrange("p (h c) -> p h c", c=64),
                        AF.Copy, scale=a_col[:, tb:tb + 1])

            # ---- stage D: attention per (head-pair, q-tile) ---------------
            attnT = attp.tile([128, 2, N], F32R)  # [hd-of-pair, pair, token]
            for j in range(2):
                for qt in range(QT):
                    ts = slice(qt * 512, (qt + 1) * 512)
                    av = [psav.tile([65, 512], F32, tag="av", name=f"av{j}_{qt}_{h}")
                          for h in range(2)]
                    for kb in range(TB):
                        ks = slice(kb * 128, (kb + 1) * 128)
                        for h in range(2):
                            hp = slice(h * 64, (h + 1) * 64)
                            ps = pssc.tile([128, 512], F32, tag="sc")
                            nc.tensor.matmul(ps[:], k_sb[hp, j, ks],
                                             q_sb[hp, j, ts],
                                             start=True, stop=True)
                            ex = expp.tile([128, 512], F32R, tag="ex")
                            if debug and j == 0 and qt == 0 and kb == 0 and h == 0:
                                dsc = expp.tile([128, 512], F32, tag="ex",
                                                name="dsc")
                                nc.vector.tensor_copy(dsc[:], ps[:])
                                nc.sync.dma_start(dbg_sc[:, :], dsc[:])
                            nc.scalar.activation(ex[:], ps[:], AF.Exp,
                                                 scale=sc_col[:, kb:kb + 1])
                            if debug and j == 0 and qt == 0 and kb == 0 and h == 0:
                                nc.sync.dma_start(dbg_ex[:, :], ex[:].bitcast(F32))
                            nc.tensor.matmul(av[h][:],
                                             v_sb[:, kb,
                                                  (2 * j + h) * 65:(2 * j + h + 1) * 65],
                                             ex[:],
                                             start=(kb == 0), stop=(kb == TB - 1))
                    if debug and j == 0 and qt == 0:
                        dav = expp.tile([65, 512], F32, tag="ex", name="dav")
                        nc.vector.tensor_copy(dav[:], av[0][:])
                        nc.sync.dma_start(dbg_av[:, :], dav[:])
                    for h in range(2):
                        rr = recp.tile([1, 512], F32, tag="rr")
                        nc.vector.reciprocal(rr[:], av[h][64:65, :])
                        rb = rbcp.tile([64, 512], F32, tag="rb")
                        nc.gpsimd.partition_broadcast(rb[:], rr[:])
                        if debug and j == 0 and qt == 0 and h == 0:
                            nc.sync.dma_start(dbg_rr[:, :], rr[:])
                            nc.sync.dma_start(dbg_rb[:, :], rb[:])
                        if h == 0:
                            nc.vector.tensor_mul(attnT[0:64, j, ts],
                                                 av[h][0:64, :], rb[:])
                        else:
                            stg = rbcp.tile([64, 512], F32R, tag="stg",
                                            name=f"stg{j}_{qt}")
                            nc.vector.tensor_mul(stg[:], av[h][0:64, :], rb[:])
                            nc.sync.dma_start(attnT[64:128, j, ts], stg[:])

            if debug:
                nc.sync.dma_start(dbg_q[:, :, :], q_sb[:].bitcast(F32))
                nc.sync.dma_start(dbg_k[:, :, :], k_sb[:].bitcast(F32))
                nc.sync.dma_start(dbg_v[:, :, :], v_sb[:].bitcast(F32))
                nc.sync.dma_start(dbg_at[:, :, :], attnT[:].bitcast(F32))
                nc.sync.dma_start(dbg_st[:, 0:TB], a_col[:])
                nc.sync.dma_start(dbg_st[:, TB:2 * TB], nmu_col[:])
                nc.sync.dma_start(dbg_st[:, 2 * TB:3 * TB], sd_col[:])
                nc.sync.dma_start(dbg_st[:, 3 * TB:4 * TB], sc_col[:])
                nc.sync.dma_start(dbg_rows[:, :], rows3[:, :])

            # ---- stage E: projection + bias + residual --------------------
            wproj_sb = wpool.tile([128, 2, C], F32R, tag="wbig")
            for j in range(2):
                nc.sync.dma_start(
                    wproj_sb[:, j, :],
                    wproj[j * 128:(j + 1) * 128, :].bitcast(F32R))
            for tb in range(TB):
                bs = slice(tb * 128, (tb + 1) * 128)
                xn_t = xnp.tile([128, C], F32, tag="xn")
                nc.sync.dma_start(xn_t[:], xn[bs, :])
                for cn in range(2):
                    cs = slice(cn * 512, (cn + 1) * 512)
                    ps = psmm.tile([128, 512], F32, tag="mm")
                    for j in range(2):
                        nc.tensor.matmul(ps[:], attnT[:, j, bs],
                                         wproj_sb[:, j, cs],
                                         start=(j == 0), stop=False)
                    nc.tensor.matmul(ps[:], ones_sb[:], bproj4_sb[:, cs],
                                     start=False, stop=True)
                    ot = outp.tile([128, 512], F32, tag="ot")
                    nc.vector.scalar_tensor_tensor(
                        ot[:], xn_t[:, cs], 0.25, ps[:],
                        op0=ALU.mult, op1=ALU.add)
                    nc.sync.dma_start(out[bs, cs], ot[:])

    nc.compile()
    return nc


# ---------------------------------------------------------------------------
# host side: shard, run (cached jit), gather

_RUNNER = None


def _make_runner():
    import jax
    from jax.sharding import Mesh, PartitionSpec
    from jax.experimental.shard_map import shard_map
    from concourse import bass2jax

    bass2jax.install_neuronx_cc_hook()
    nc = build_nc()

    partition_name = (nc.partition_id_tensor.name
                      if nc.partition_id_tensor else None)
    in_names, out_names, out_avals = [], [], []
    for alloc in nc.m.functions[0].allocations:
        if not isinstance(alloc, mybir.MemoryLocationSet):
            continue
        if not alloc.memorylocations:
            continue
        name = alloc.memorylocations[0].name
        if alloc.kind == "ExternalInput":
            if name != partition_name:
                in_names.append(name)
        elif alloc.kind == "ExternalOutput":
            out_names.append(name)
            out_avals.append(jax.core.ShapedArray(
                tuple(alloc.tensor_shape), mybir.dt.np(alloc.dtype)))
    n_params = len(in_names)
    all_names = tuple(in_names + out_names
                      + ([partition_name] if partition_name else []))

    def _body(*args):
        operands = list(args)
        if partition_name is not None:
            operands.append(bass2jax.partition_id_tensor())
        outs = bass2jax._bass_exec_p.bind(
            *operands,
            out_avals=tuple(out_avals),
            in_names=all_names,
            out_names=tuple(out_names),
            lowering_input_output_aliases=(),
            sim_require_finite=True,
            sim_require_nnan=True,
            nc=nc,
        )
        return tuple(outs)

    devices = jax.devices()[:NCORES]
    mesh = Mesh(np.asarray(devices), ("core",))
    n_outs = len(out_names)
    donate = tuple(range(n_params, n_params + n_outs))
    sharded = jax.jit(
        shard_map(_body, mesh=mesh,
                  in_specs=(PartitionSpec("core"),) * (n_params + n_outs),
                  out_specs=(PartitionSpec("core"),) * n_outs,
                  check_rep=False),
        donate_argnums=donate, keep_unused=True)

    def run(in_maps):
        concat_in = [
            np.concatenate([np.asarray(in_maps[c][k]) for c in range(NCORES)],
                           axis=0)
            for k in in_names
        ]
        zeros = [np.zeros((NCORES * a.shape[0], *a.shape[1:]), a.dtype)
                 for a in out_avals]
        out_arrs = sharded(*concat_in, *zeros)
        import jax as _jax
        out_arrs = _jax.block_until_ready(out_arrs)
        return [
            {name: np.asarray(out_arrs[i]).reshape(
                NCORES, *out_avals[i].shape)[c]
             for i, name in enumerate(out_names)}
            for c in range(NCORES)
        ]

    return run


def get_runner():
    global _RUNNER
    if _RUNNER is None:
        _RUNNER = _make_runner()
    return _RUNNER


def make_in_maps(x, w_qkv, w_proj, b_proj, ln_gamma, ln_beta):
    x = np.asarray(x, np.float32)
    w_qkv = np.asarray(w_qkv, np.float32)
    w_proj = np.asarray(w_proj, np.float32)
    b_proj = np.asarray(b_proj, np.float32)
    ln_gamma = np.asarray(ln_gamma, np.float32)
    ln_beta = np.asarray(ln_beta, np.float32)

    Wp = w_qkv * ln_gamma[:, None]          # diag(gamma) @ W
    wsum = Wp.sum(axis=0)                    # [3C]
    bW = ln_beta @ w_qkv                     # [3C]

    in_maps = []
    for c in range(NCORES):
        b, hg = divmod(c, HG)
        qs = slice(hg * CORE_HD, (hg + 1) * CORE_HD)
        ks = slice(C + hg * CORE_HD, C + (hg + 1) * CORE_HD)
        vs = slice(2 * C + hg * CORE_HD, 2 * C + (hg + 1) * CORE_HD)
        wqk_c = np.ascontiguousarray(
            np.concatenate([Wp[:, qs], Wp[:, ks]], axis=1))
        in_maps.append({
            "xT": np.ascontiguousarray(x[b].T),
            "xn": np.ascontiguousarray(x[b]),
            "wqk": wqk_c,
            "wv": np.ascontiguousarray(Wp[:, vs]),
            "wqksum": np.concatenate([wsum[qs], wsum[ks]])[None, :].copy(),
            "wqkbias": np.concatenate([bW[qs], bW[ks]])[None, :].copy(),
            "wvsum": wsum[vs][None, :].copy(),
            "wvbias": bW[vs][None, :].copy(),
            "wproj": np.ascontiguousarray(w_proj[hg * CORE_HD:(hg + 1) * CORE_HD, :]),
            "bproj4": 0.25 * b_proj[None, :],
            "onesr": np.ones((1, 128), np.float32),
            "onesv": np.ones((128, TB, 4, 1), np.float32),
        })
    return in_maps


def gather(results):
    out = np.zeros((B, N, C), np.float32)
    for c in range(NCORES):
        b = c // HG
        out[b] += results[c]["out"]
    return out


def kernel(x, w_qkv, w_proj, b_proj, ln_gamma, ln_beta):
    run = get_runner()
    in_maps = make_in_maps(x, w_qkv, w_proj, b_proj, ln_gamma, ln_beta)
    results = run(in_maps)
    return gather(results)


# revision 16
# speedup vs baseline: 3530.3193x; 3530.3193x over previous
"""Fused MultiHeadAttention block (LN -> QKV -> attention -> proj -> +residual)
for Trainium2, sharded over 8 NeuronCores.

Sharding: data parallel over batch (B=2) x tensor parallel over heads
(16 heads -> 4 groups of 4). Core c handles batch c//4, head group c%4.
Each core computes a partial projection output (contraction over its 256
hd-dims) plus 0.25*(x + b_proj); the host sums the 4 partials per batch.

Math notes (all LN folding is exact linear algebra, valid for any
gamma/beta):
  h = LN(x)*gamma + beta;  qkv = h @ W
  With W' = diag(gamma) @ W, bW = beta @ W, rs[t] = 1/sqrt(var[t]+eps):
    qkv[t,d] = rs[t] * ( (x@W')[t,d] + (-mu[t])*colsum(W')[d]
                         + sd[t]*bW[d] ),   sd[t] = 1/rs[t]
  The two rank-1 terms are added in PSUM via K=1 matmuls; the rs[t] scale
  is applied at PSUM->SBUF drain time (per-partition for V whose rows are
  tokens; via a broadcast tile for Q; folded into the softmax exp scale
  for K, since exp(score_true) = exp(0.125*rs_k * psum_score) when Q
  already carries its rs factor).

  Softmax skips the max-subtraction (scores here are O(1), exp is safe);
  denominators come for free as row 64 of the AV matmul by appending a
  ones column to V (lhsT [k,65] -> out rows 0..63 = attn@V, row 64 =
  sum(exp)). Normalization folds into the attention-output tiles before
  the projection matmul.

Matmuls run in float32r (full PE rate at free-dim>=256, ~1e-3 rel err);
everything else is f32.
"""

import numpy as np

import concourse.bass as bass
import concourse.tile as tile
from concourse import bacc, mybir

F32 = mybir.dt.float32
F32R = mybir.dt.float32r
AF = mybir.ActivationFunctionType
ALU = mybir.AluOpType

B, N, C = 2, 2048, 1024
HEADS, HD = 16, 64
NCORES = 8
HG = 4                      # head groups (tensor-parallel degree)
H_PER = HEADS // HG         # 4 heads per core
CORE_HD = H_PER * HD        # 256 hd-dims per core
KT = C // 128               # 8 contraction tiles over channels
QT = 4                      # token tiles of 512
TB = N // 128               # 16 token blocks of 128
EPS = 1e-5


def build_nc(debug=False, repeat=1):
    nc = bacc.Bacc("TRN2", target_bir_lowering=False, debug=False,
                   num_devices=NCORES)

    xT = nc.dram_tensor("xT", [C, N], F32, kind="ExternalInput")
    xn = nc.dram_tensor("xn", [N, C], F32, kind="ExternalInput")
    wqk = nc.dram_tensor("wqk", [C, 512], F32, kind="ExternalInput")
    wv = nc.dram_tensor("wv", [C, 256], F32, kind="ExternalInput")
    wqksum = nc.dram_tensor("wqksum", [1, 512], F32, kind="ExternalInput")
    wqkbias = nc.dram_tensor("wqkbias", [1, 512], F32, kind="ExternalInput")
    wvsum = nc.dram_tensor("wvsum", [1, 256], F32, kind="ExternalInput")
    wvbias = nc.dram_tensor("wvbias", [1, 256], F32, kind="ExternalInput")
    wproj = nc.dram_tensor("wproj", [CORE_HD, C], F32, kind="ExternalInput")
    bproj4 = nc.dram_tensor("bproj4", [1, C], F32, kind="ExternalInput")
    onesr = nc.dram_tensor("onesr", [1, 128], F32, kind="ExternalInput")
    onesv = nc.dram_tensor("onesv", [128, TB, 4, 1], F32, kind="ExternalInput")
    out = nc.dram_tensor("out", [N, C], F32, kind="ExternalOutput")
    if debug:
        dbg_q = nc.dram_tensor("dbg_q", [128, 2, N], F32, kind="ExternalOutput")
        dbg_k = nc.dram_tensor("dbg_k", [128, 2, N], F32, kind="ExternalOutput")
        dbg_v = nc.dram_tensor("dbg_v", [128, TB, 260], F32, kind="ExternalOutput")
        dbg_at = nc.dram_tensor("dbg_at", [128, 2, N], F32, kind="ExternalOutput")
        dbg_st = nc.dram_tensor("dbg_st", [128, 4 * TB], F32, kind="ExternalOutput")
        dbg_rows = nc.dram_tensor("dbg_rows", [3, N], F32, kind="ExternalOutput")

    with tile.TileContext(nc) as tc:
        with (
            tc.tile_pool(name="consts", bufs=1) as consts,
            tc.tile_pool(name="wpool", bufs=1) as wpool,
            tc.tile_pool(name="xtp", bufs=2) as xtp,
            tc.tile_pool(name="xnp", bufs=2) as xnp,
            tc.tile_pool(name="stats", bufs=1) as stats,
            tc.tile_pool(name="mvp", bufs=4) as mvp,
            tc.tile_pool(name="qkv", bufs=1) as qkvp,
            tc.tile_pool(name="att", bufs=1) as attp,
            tc.tile_pool(name="exps", bufs=3) as expp,
            tc.tile_pool(name="rec", bufs=2) as recp,
            tc.tile_pool(name="rbc", bufs=3) as rbcp,
            tc.tile_pool(name="outs", bufs=2) as outp,
            tc.tile_pool(name="dram", bufs=1, space="DRAM") as dramp,
            tc.tile_pool(name="psmm", bufs=2, space="PSUM") as psmm,
            tc.tile_pool(name="pssc", bufs=3, space="PSUM") as pssc,
            tc.tile_pool(name="psav", bufs=3, space="PSUM") as psav,
        ):
            # ---- constants / weights (loaded once, outside repeat) --------
            wqk_sb = wpool.tile([128, KT, 512], F32R, tag="wbig")
            for k in range(KT):
                nc.sync.dma_start(
                    wqk_sb[:, k, :],
                    wqk[k * 128:(k + 1) * 128, :].bitcast(F32R))
            wv_sb = consts.tile([128, KT, 256], F32R)
            for k in range(KT):
                nc.sync.dma_start(
                    wv_sb[:, k, :],
                    wv[k * 128:(k + 1) * 128, :].bitcast(F32R))
            wqksum_sb = consts.tile([1, 512], F32R)
            nc.sync.dma_start(wqksum_sb[:], wqksum[:, :].bitcast(F32R))
            wqkbias_sb = consts.tile([1, 512], F32R)
            nc.sync.dma_start(wqkbias_sb[:], wqkbias[:, :].bitcast(F32R))
            wvsum_sb = consts.tile([1, 256], F32R)
            nc.sync.dma_start(wvsum_sb[:], wvsum[:, :].bitcast(F32R))
            wvbias_sb = consts.tile([1, 256], F32R)
            nc.sync.dma_start(wvbias_sb[:], wvbias[:, :].bitcast(F32R))
            bproj4_sb = consts.tile([1, C], F32R)
            nc.sync.dma_start(bproj4_sb[:], bproj4[:, :].bitcast(F32R))
            ones_sb = consts.tile([1, 128], F32R)
            nc.sync.dma_start(ones_sb[:], onesr[:, :].bitcast(F32R))
            eps_sb = consts.tile([128, 1], F32)
            nc.vector.memset(eps_sb[:], EPS)
            wproj_sb = wpool.tile([128, 2, C], F32R, tag="wproj", name="wproj_sb")
            for j in range(2):
                nc.sync.dma_start(
                    wproj_sb[:, j, :],
                    wproj[j * 128:(j + 1) * 128, :].bitcast(F32R))

            def compute():
                # ---- stage A: LN stats from natural-layout x --------------
                a_col = stats.tile([128, TB], F32, name="a_col", tag="a_col")
                sd_col = stats.tile([128, TB], F32, name="sd_col", tag="sd_col")
                nmu_col = stats.tile([128, TB], F32, name="nmu_col", tag="nmu_col")
                sc_col = stats.tile([128, TB], F32, name="sc_col", tag="sc_col")
                for tb in range(TB):
                    xn_t = xnp.tile([128, C], F32, tag="xn", name=f"xna{tb}")
                    nc.sync.dma_start(xn_t[:], xn[tb * 128:(tb + 1) * 128, :])
                    st = mvp.tile([128, 2, 6], F32, tag="bnst", name=f"st{tb}")
                    xg = xn_t[:].rearrange("p (g d) -> p g d", g=2)
                    for g in range(2):
                        nc.vector.bn_stats(st[:, g, :], xg[:, g, :])
                    mv = mvp.tile([128, 2], F32, tag="bnmv", name=f"mv{tb}")
                    nc.vector.bn_aggr(mv[:], st[:])
                    nc.scalar.activation(sd_col[:, tb:tb + 1], mv[:, 1:2],
                                         AF.Sqrt, bias=eps_sb[:])
                    nc.vector.reciprocal(a_col[:, tb:tb + 1],
                                         sd_col[:, tb:tb + 1])
                    nc.vector.tensor_scalar_mul(nmu_col[:, tb:tb + 1],
                                                mv[:, 0:1], -1.0)
                nc.vector.tensor_scalar_mul(sc_col[:], a_col[:], 0.125)

                # cols -> DRAM rows -> row/broadcast tiles (per q-tile so
                # stage B's rank-1 matmuls only wait on 1/4 of the stats)
                rows3 = dramp.tile([3, N], F32, name="rows3", tag="rows3")
                nmu_row = stats.tile([1, N], F32R, name="nmu_row", tag="nmu_row")
                sd_row = stats.tile([1, N], F32R, name="sd_row", tag="sd_row")
                a_bc = stats.tile([128, N], F32, name="a_bc", tag="a_bc")
                for qt in range(QT):
                    ts = slice(qt * 512, (qt + 1) * 512)
                    t4 = slice(qt * 4, (qt + 1) * 4)
                    for i, col in enumerate((a_col, nmu_col, sd_col)):
                        nc.sync.dma_start(
                            rows3[i:i + 1, ts].rearrange(
                                "o (t p) -> o p t", p=128),
                            col[:, t4])
                    nc.sync.dma_start(nmu_row[:, ts],
                                      rows3[1:2, ts].bitcast(F32R))
                    nc.sync.dma_start(sd_row[:, ts],
                                      rows3[2:3, ts].bitcast(F32R))
                    nc.sync.dma_start(a_bc[:, ts],
                                      rows3[0:1, ts].partition_broadcast(128))

                # ---- stages B/C: QKV projections --------------------------
                q_sb = qkvp.tile([128, 2, N], F32R, name="q_sb", tag="q_sb")
                k_sb = qkvp.tile([128, 2, N], F32R, name="k_sb", tag="k_sb")
                v_sb = qkvp.tile([128, TB, 4 * 65], F32R, name="v_sb", tag="v_sb")
                v_view = v_sb[:].rearrange("p t (h c) -> p t h c", c=65)
                nc.sync.dma_start(v_view[:, :, :, 64:65],
                                  onesv[:, :, :, :].bitcast(F32R))

                for qt in range(QT):
                    ts = slice(qt * 512, (qt + 1) * 512)
                    xT_t = xtp.tile([128, KT, 512], F32R, tag="xT",
                                    name=f"xT{qt}")
                    for k in range(KT):
                        nc.sync.dma_start(
                            xT_t[:, k, :],
                            xT[k * 128:(k + 1) * 128, ts].bitcast(F32R))
                    # Q/K: out [128 dims(2 heads), 512 tokens]
                    for dblk in range(4):
                        ds = slice(dblk * 128, (dblk + 1) * 128)
                        ps = psmm.tile([128, 512], F32, tag="mm",
                                       name=f"qk{qt}_{dblk}")
                        for k in range(KT):
                            nc.tensor.matmul(ps[:], wqk_sb[:, k, ds],
                                             xT_t[:, k, :],
                                             start=(k == 0), stop=False)
                        nc.tensor.matmul(ps[:], wqksum_sb[:, ds],
                                         nmu_row[:, ts],
                                         start=False, stop=False)
                        nc.tensor.matmul(ps[:], wqkbias_sb[:, ds],
                                         sd_row[:, ts],
                                         start=False, stop=True)
                        dst = q_sb if dblk < 2 else k_sb
                        nc.vector.tensor_mul(dst[:, dblk % 2, ts], ps[:],
                                             a_bc[:, ts])
                    # V: out [128 tokens, 256 dims]
                    for t4 in range(4):
                        tb = qt * 4 + t4
                        bs = slice(tb * 128, (tb + 1) * 128)
                        ps = psmm.tile([128, 256], F32, tag="mm",
                                       name=f"v{qt}_{t4}")
                        for k in range(KT):
                            nc.tensor.matmul(
                                ps[:], xT_t[:, k, t4 * 128:(t4 + 1) * 128],
                                wv_sb[:, k, :], start=(k == 0), stop=False)
                        nc.tensor.matmul(ps[:], nmu_row[:, bs], wvsum_sb[:],
                                         start=False, stop=False)
                        nc.tensor.matmul(ps[:], sd_row[:, bs], wvbias_sb[:],
                                         start=False, stop=True)
                        nc.scalar.activation(
                            v_view[:, tb, :, 0:64],
                            ps[:].rearrange("p (h c) -> p h c", c=64),
                            AF.Copy, scale=a_col[:, tb:tb + 1])

                # ---- stage D: attention per (head-pair, q-tile) -----------
                attnT = attp.tile([128, 2, N], F32R, name="attnT", tag="attnT")
                for j in range(2):
                    for qt in range(QT):
                        ts = slice(qt * 512, (qt + 1) * 512)
                        av = [psav.tile([65, 512], F32, tag="av",
                                        name=f"av{j}_{qt}_{h}")
                              for h in range(2)]
                        for kb in range(TB):
                            ks = slice(kb * 128, (kb + 1) * 128)
                            for h in range(2):
                                hp = slice(h * 64, (h + 1) * 64)
                                ps = pssc.tile([128, 512], F32, tag="sc",
                                               name=f"sc{j}_{qt}_{kb}_{h}")
                                nc.tensor.matmul(ps[:], k_sb[hp, j, ks],
                                                 q_sb[hp, j, ts],
                                                 start=True, stop=True)
                                ex = expp.tile([128, 512], F32R, tag="ex",
                                               name=f"ex{j}_{qt}_{kb}_{h}")
                                nc.scalar.activation(ex[:], ps[:], AF.Exp,
                                                     scale=sc_col[:, kb:kb + 1])
                                nc.tensor.matmul(
                                    av[h][:],
                                    v_sb[:, kb,
                                         (2 * j + h) * 65:(2 * j + h + 1) * 65],
                                    ex[:],
                                    start=(kb == 0), stop=(kb == TB - 1))
                        for h in range(2):
                            rr = recp.tile([1, 512], F32, tag="rr",
                                           name=f"rr{j}_{qt}_{h}")
                            nc.vector.reciprocal(rr[:], av[h][64:65, :])
                            rb = rbcp.tile([64, 512], F32, tag="rb",
                                           name=f"rb{j}_{qt}_{h}")
                            nc.gpsimd.partition_broadcast(rb[:], rr[:])
                            if h == 0:
                                nc.vector.tensor_mul(attnT[0:64, j, ts],
                                                     av[h][0:64, :], rb[:])
                            else:
                                stg = rbcp.tile([64, 512], F32R, tag="stg",
                                                name=f"stg{j}_{qt}")
                                nc.vector.tensor_mul(stg[:], av[h][0:64, :],
                                                     rb[:])
                                nc.sync.dma_start(attnT[64:128, j, ts], stg[:])

                if debug:
                    nc.sync.dma_start(dbg_q[:, :, :], q_sb[:].bitcast(F32))
                    nc.sync.dma_start(dbg_k[:, :, :], k_sb[:].bitcast(F32))
                    nc.sync.dma_start(dbg_v[:, :, :], v_sb[:].bitcast(F32))
                    nc.sync.dma_start(dbg_at[:, :, :], attnT[:].bitcast(F32))
                    nc.sync.dma_start(dbg_st[:, 0:TB], a_col[:])
                    nc.sync.dma_start(dbg_st[:, TB:2 * TB], nmu_col[:])
                    nc.sync.dma_start(dbg_st[:, 2 * TB:3 * TB], sd_col[:])
                    nc.sync.dma_start(dbg_st[:, 3 * TB:4 * TB], sc_col[:])
                    nc.sync.dma_start(dbg_rows[:, :], rows3[:, :])

                # ---- stage E: projection + bias + residual ----------------
                for tb in range(TB):
                    bs = slice(tb * 128, (tb + 1) * 128)
                    xn_t = xnp.tile([128, C], F32, tag="xne", name=f"xne{tb}")
                    nc.sync.dma_start(xn_t[:], xn[bs, :])
                    for cn in range(2):
                        cs = slice(cn * 512, (cn + 1) * 512)
                        ps = psmm.tile([128, 512], F32, tag="mm",
                                       name=f"pj{tb}_{cn}")
                        for j in range(2):
                            nc.tensor.matmul(ps[:], attnT[:, j, bs],
                                             wproj_sb[:, j, cs],
                                             start=(j == 0), stop=False)
                        nc.tensor.matmul(ps[:], ones_sb[:], bproj4_sb[:, cs],
                                         start=False, stop=True)
                        ot = outp.tile([128, 512], F32, tag="ot",
                                       name=f"ot{tb}_{cn}")
                        nc.vector.scalar_tensor_tensor(
                            ot[:], xn_t[:, cs], 0.25, ps[:],
                            op0=ALU.mult, op1=ALU.add)
                        nc.sync.dma_start(out[bs, cs], ot[:])

            if repeat == 1:
                compute()
            else:
                with tc.For_i(0, repeat):
                    compute()

    nc.compile()
    return nc


# ---------------------------------------------------------------------------
# host side: shard, run (cached jit), gather

_RUNNERS = {}


def _make_runner(repeat=1):
    import jax
    from jax.sharding import Mesh, PartitionSpec
    from jax.experimental.shard_map import shard_map
    from concourse import bass2jax

    bass2jax.install_neuronx_cc_hook()
    nc = build_nc(repeat=repeat)

    partition_name = (nc.partition_id_tensor.name
                      if nc.partition_id_tensor else None)
    in_names, out_names, out_avals = [], [], []
    for alloc in nc.m.functions[0].allocations:
        if not isinstance(alloc, mybir.MemoryLocationSet):
            continue
        if not alloc.memorylocations:
            continue
        name = alloc.memorylocations[0].name
        if alloc.kind == "ExternalInput":
            if name != partition_name:
                in_names.append(name)
        elif alloc.kind == "ExternalOutput":
            out_names.append(name)
            out_avals.append(jax.core.ShapedArray(
                tuple(alloc.tensor_shape), mybir.dt.np(alloc.dtype)))
    n_params = len(in_names)
    all_names = tuple(in_names + out_names
                      + ([partition_name] if partition_name else []))

    def _body(*args):
        operands = list(args)
        if partition_name is not None:
            operands.append(bass2jax.partition_id_tensor())
        outs = bass2jax._bass_exec_p.bind(
            *operands,
            out_avals=tuple(out_avals),
            in_names=all_names,
            out_names=tuple(out_names),
            lowering_input_output_aliases=(),
            sim_require_finite=True,
            sim_require_nnan=True,
            nc=nc,
        )
        return tuple(outs)

    devices = jax.devices()[:NCORES]
    mesh = Mesh(np.asarray(devices), ("core",))
    n_outs = len(out_names)
    donate = tuple(range(n_params, n_params + n_outs))
    sharded = jax.jit(
        shard_map(_body, mesh=mesh,
                  in_specs=(PartitionSpec("core"),) * (n_params + n_outs),
                  out_specs=(PartitionSpec("core"),) * n_outs,
                  check_rep=False),
        donate_argnums=donate, keep_unused=True)

    def run(in_maps):
        concat_in = [
            np.concatenate([np.asarray(in_maps[c][k]) for c in range(NCORES)],
                           axis=0)
            for k in in_names
        ]
        zeros = [np.zeros((NCORES * a.shape[0], *a.shape[1:]), a.dtype)
                 for a in out_avals]
        out_arrs = sharded(*concat_in, *zeros)
        import jax as _jax
        out_arrs = _jax.block_until_ready(out_arrs)
        return [
            {name: np.asarray(out_arrs[i]).reshape(
                NCORES, *out_avals[i].shape)[c]
             for i, name in enumerate(out_names)}
            for c in range(NCORES)
        ]

    return run


def get_runner(repeat=1):
    if repeat not in _RUNNERS:
        _RUNNERS[repeat] = _make_runner(repeat=repeat)
    return _RUNNERS[repeat]


def make_in_maps(x, w_qkv, w_proj, b_proj, ln_gamma, ln_beta):
    x = np.asarray(x, np.float32)
    w_qkv = np.asarray(w_qkv, np.float32)
    w_proj = np.asarray(w_proj, np.float32)
    b_proj = np.asarray(b_proj, np.float32)
    ln_gamma = np.asarray(ln_gamma, np.float32)
    ln_beta = np.asarray(ln_beta, np.float32)

    Wp = w_qkv * ln_gamma[:, None]          # diag(gamma) @ W
    wsum = Wp.sum(axis=0)                    # [3C]
    bW = ln_beta @ w_qkv                     # [3C]

    in_maps = []
    for c in range(NCORES):
        b, hg = divmod(c, HG)
        qs = slice(hg * CORE_HD, (hg + 1) * CORE_HD)
        ks = slice(C + hg * CORE_HD, C + (hg + 1) * CORE_HD)
        vs = slice(2 * C + hg * CORE_HD, 2 * C + (hg + 1) * CORE_HD)
        wqk_c = np.ascontiguousarray(
            np.concatenate([Wp[:, qs], Wp[:, ks]], axis=1))
        in_maps.append({
            "xT": np.ascontiguousarray(x[b].T),
            "xn": np.ascontiguousarray(x[b]),
            "wqk": wqk_c,
            "wv": np.ascontiguousarray(Wp[:, vs]),
            "wqksum": np.concatenate([wsum[qs], wsum[ks]])[None, :].copy(),
            "wqkbias": np.concatenate([bW[qs], bW[ks]])[None, :].copy(),
            "wvsum": wsum[vs][None, :].copy(),
            "wvbias": bW[vs][None, :].copy(),
            "wproj": np.ascontiguousarray(w_proj[hg * CORE_HD:(hg + 1) * CORE_HD, :]),
            "bproj4": 0.25 * b_proj[None, :],
            "onesr": np.ones((1, 128), np.float32),
            "onesv": np.ones((128, TB, 4, 1), np.float32),
        })
    return in_maps


def gather(results):
    out = np.zeros((B, N, C), np.float32)
    for c in range(NCORES):
        b = c // HG
        out[b] += results[c]["out"]
    return out


def kernel(x, w_qkv, w_proj, b_proj, ln_gamma, ln_beta):
    run = get_runner()
    in_maps = make_in_maps(x, w_qkv, w_proj, b_proj, ln_gamma, ln_beta)
    results = run(in_maps)
    return gather(results)
